# revision 18
# baseline (speedup 1.0000x reference)
import sys

for _p in ('/opt/trn_rl_repo', '/root/.axon_site'):
    if _p not in sys.path:
        sys.path.insert(0, _p)

import numpy as np

B, H, W = 8, 512, 512
K = 3
NCORES = 8
# Row-blocked layout: partition p holds output rows 4p..4p+3 (j in 0..3).
# Extended map tiles carry 6 row-variants per partition (jj = j + ky + 1,
# ky in {-1,0,1}) so every vertical shift is a free-dim view.
JB = 4            # rows per partition
JJ = 6            # extended rows per partition (j + ky + 1, 0..5)
AW = 520          # padded image row width
DW = 516          # difference-map row width

_compiled = None


def _build():
    import concourse.bacc as bacc
    import concourse.mybir as mybir
    from concourse.tile import TileContext

    f16, f32 = mybir.dt.float16, mybir.dt.float32
    ALU = mybir.AluOpType
    ACTF = mybir.ActivationFunctionType

    nc = bacc.Bacc("TRN2", target_bir_lowering=False, debug=False,
                   num_devices=NCORES)
    ae_d = nc.dram_tensor("ae", [128, JJ, AW], f16, kind="ExternalInput")
    dxe_d = nc.dram_tensor("dxe", [128, JJ, DW], f16, kind="ExternalInput")
    dye_d = nc.dram_tensor("dye", [128, JJ, DW], f16, kind="ExternalInput")
    dxye_d = nc.dram_tensor("dxye", [128, JJ, DW], f16, kind="ExternalInput")
    off = nc.dram_tensor("off", [2 * K * K, H, W], f16, kind="ExternalInput")
    wdg = nc.dram_tensor("wdg", [128, K * K, 128], f16, kind="ExternalInput")
    out = nc.dram_tensor("out", [H, W], f16, kind="ExternalOutput")

    with TileContext(nc) as tc:
        with (
            tc.tile_pool(name="maps", bufs=1) as mp,
            tc.tile_pool(name="lxy", bufs=6) as lp,
            tc.tile_pool(name="tmp", bufs=2) as tp,
            tc.tile_pool(name="cst", bufs=1) as cp,
            tc.tile_pool(name="psum", bufs=1, space="PSUM") as pp,
        ):
            psum = pp.tile([128, JB, W], f32, name="psum")

            # Loads go out in exact DVE-consumption order, alternating the
            # two HWDGE rings so the stream stays just ahead of compute.
            ring = [nc.sync, nc.scalar]
            rix = [0]

            def dma(dst, src):
                eng = ring[rix[0] & 1]
                rix[0] += 1
                eng.dma_start(out=dst, in_=src)

            def load_ch(k, d):
                # d=1 -> lx (x offsets), d=0 -> ly
                t = lp.tile([128, JB, W], f16, tag=f"l{d}", name=f"l{d}{k}")
                dma(t[:], off[2 * k + d].rearrange("(p j) c -> p j c", j=JB))
                return t

            def map_tile(name, w):
                return mp.tile([128, JJ, w], f16, name=name)

            def load_ch_half(k, d, t, js):
                dma(t[:, js],
                    off[2 * k + d].rearrange("(p j) c -> p j c", j=JB)[:, js])

            # loads split fine-grained at the front so the very first DVE
            # ops wait on as few bytes as possible
            lx0 = lp.tile([128, JB, W], f16, tag="l1", name="lx0")
            ly0 = lp.tile([128, JB, W], f16, tag="l0", name="ly0")
            dxe = map_tile("dxe", DW)
            dxye = map_tile("dxye", DW)
            dye = map_tile("dye", DW)
            load_ch_half(0, 1, lx0, slice(0, 2))
            dma(dxe[:, 0:2], dxe_d[:, 0:2])
            wd = cp.tile([128, K * K, 128], f16, name="wd")
            dma(wd[:], wdg[:])
            dma(dxye[:, 0:2], dxye_d[:, 0:2])
            dma(dye[:, 0:2], dye_d[:, 0:2])
            load_ch_half(0, 0, ly0, slice(0, 2))
            load_ch_half(0, 1, lx0, slice(2, 4))
            dma(dxe[:, 2:4], dxe_d[:, 2:4])
            dma(dxye[:, 2:4], dxye_d[:, 2:4])
            dma(dye[:, 2:4], dye_d[:, 2:4])
            load_ch_half(0, 0, ly0, slice(2, 4))
            lx1 = load_ch(1, 1)
            ly1 = load_ch(1, 0)
            lx2 = load_ch(2, 1)
            ly2 = load_ch(2, 0)
            dma(dxe[:, 4:6], dxe_d[:, 4:6])
            dma(dxye[:, 4:6], dxye_d[:, 4:6])
            dma(dye[:, 4:6], dye_d[:, 4:6])
            lxs = {0: lx0, 1: lx1, 2: lx2}
            lys = {0: ly0, 1: ly1, 2: ly2}
            lxs[3] = load_ch(3, 1)
            lys[3] = load_ch(3, 0)
            ae = map_tile("ae", AW)
            dma(ae[:], ae_d[:])
            for k in range(4, K * K):
                lxs[k] = load_ch(k, 1)
                lys[k] = load_ch(k, 0)

            first = [True] * JB

            def mm(j, wk, mov, stop=False):
                nc.tensor.matmul(psum[:, j, :], wk, mov,
                                 start=first[j], stop=stop)
                first[j] = False

            for k in range(K * K):
                r3, kc = divmod(k, K)
                lx, ly = lxs.pop(k), lys.pop(k)
                wk = wd[:, k, :]
                if k in (0, K * K - 1):
                    # first tap: halves let compute start on fewer bytes;
                    # final tap: halves let banks 0/1 close and drain early
                    for j0, j1 in ((0, 1), (2, 3)):
                        js = slice(j0, j1 + 1)
                        dxv = dxe[:, r3 + j0:r3 + j1 + 1, kc:kc + W]
                        dxyv = dxye[:, r3 + j0:r3 + j1 + 1, kc:kc + W]
                        dyv = dye[:, r3 + j0:r3 + j1 + 1, kc:kc + W]
                        t = tp.tile([128, 2, W], f16, tag="ht", name="ht")
                        t3 = tp.tile([128, 2, W], f16, tag="ht3", name="ht3")
                        s = tp.tile([128, 2, W], f16, tag="hs", name="hs")
                        t2 = tp.tile([128, 2, W], f16, tag="ht2", name="ht2")
                        nc.vector.tensor_tensor(t[:], lx[:, js], dxv, ALU.mult)
                        nc.vector.tensor_tensor(t3[:], lx[:, js], dxyv,
                                                ALU.mult)
                        nc.vector.tensor_tensor(s[:], t3[:], dyv, ALU.add)
                        nc.vector.tensor_tensor(t2[:], ly[:, js], s[:],
                                                ALU.mult)
                        stop = k == K * K - 1
                        if stop:
                            for j in (j0, j1):
                                mm(j, wk, ae[:, j + r3, kc:kc + W])
                        for j in (j0, j1):
                            mm(j, wk, t[:, j - j0, :])
                        for j in (j0, j1):
                            mm(j, wk, t2[:, j - j0, :], stop=stop)
                        if not stop:
                            for j in (j0, j1):
                                mm(j, wk, ae[:, j + r3, kc:kc + W])
                    continue
                if True:
                    dxv = dxe[:, r3:r3 + JB, kc:kc + W]
                    dxyv = dxye[:, r3:r3 + JB, kc:kc + W]
                    dyv = dye[:, r3:r3 + JB, kc:kc + W]
                    t = tp.tile([128, JB, W], f16, tag="t", name="t", bufs=3)
                    t3 = tp.tile([128, JB, W], f16, tag="t3", name="t3", bufs=3)
                    s = tp.tile([128, JB, W], f16, tag="s", name="s", bufs=3)
                    t2 = tp.tile([128, JB, W], f16, tag="t2", name="t2", bufs=3)
                    nc.vector.tensor_tensor(t[:], lx[:], dxv, ALU.mult)
                    nc.vector.tensor_tensor(t3[:], lx[:], dxyv, ALU.mult)
                    nc.vector.tensor_tensor(s[:], t3[:], dyv, ALU.add)
                    nc.vector.tensor_tensor(t2[:], ly[:], s[:], ALU.mult)
                    for j in range(JB):
                        mm(j, wk, t[:, j, :])
                    for j in range(JB):
                        mm(j, wk, t2[:, j, :])
                    for j in range(JB):
                        mm(j, wk, ae[:, j + r3, kc:kc + W])
            # tail: DVE casts banks 0/1 in parallel with ACT copying 2/3;
            # two output DMAs on the two rings.
            res01 = cp.tile([128, 2, W], f16, name="res01")
            res23 = cp.tile([128, 2, W], f16, name="res23")
            nc.vector.tensor_copy(res01[:], psum[:, 0:2, :])
            nc.scalar.activation(res23[:], psum[:, 2:4, :], ACTF.Copy)
            outv = out.rearrange("(p j) c -> p j c", j=JB)
            nc.sync.dma_start(out=outv[:, 0:2], in_=res01[:])
            nc.scalar.dma_start(out=outv[:, 2:4], in_=res23[:])

    nc.compile()
    return nc


def kernel(input, weight, offset):
    global _compiled
    from concourse.bass_utils import run_bass_kernel_spmd

    if _compiled is None:
        _compiled = _build()
    nc = _compiled

    input = np.asarray(input, dtype=np.float32)
    offset = np.asarray(offset, dtype=np.float32)
    w9 = np.asarray(weight, dtype=np.float32).reshape(K * K)
    wdg = np.zeros((128, K * K, 128), np.float16)
    idx = np.arange(128)
    for k in range(K * K):
        wdg[idx, k, idx] = w9[k].astype(np.float16)

    jj_rows = 4 * np.arange(128)[:, None] + np.arange(JJ)[None, :]

    in_maps = []
    for b in range(B):
        ipad = np.zeros((515, AW), np.float32)
        ipad[1:H + 1, 1:W + 1] = input[b]
        dx = ipad[:, 1:] - ipad[:, :-1]          # [515, 519]
        dy = ipad[1:, :] - ipad[:-1, :]          # [514, 520]
        dxy = dy[:, 1:] - dy[:, :-1]             # [514, 519]
        ae = ipad.astype(np.float16)[jj_rows]                  # [128,6,520]
        dxe = np.ascontiguousarray(dx[:, :DW].astype(np.float16)[jj_rows])
        dye = np.ascontiguousarray(dy[:, :DW].astype(np.float16)[jj_rows])
        dxye = np.ascontiguousarray(dxy[:, :DW].astype(np.float16)[jj_rows])
        offh = np.ascontiguousarray(offset[b].astype(np.float16))
        in_maps.append({
            "ae": np.ascontiguousarray(ae),
            "dxe": dxe, "dye": dye, "dxye": dxye,
            "off": offh, "wdg": wdg,
        })

    res = run_bass_kernel_spmd(nc, in_maps, list(range(NCORES)), trace=False)
    return np.stack([res.results[b]["out"] for b in range(B)],
                    axis=0).astype(np.float32)


# revision 19
# speedup vs baseline: 1.0403x; 1.0403x over previous
import sys

for _p in ('/opt/trn_rl_repo', '/root/.axon_site'):
    if _p not in sys.path:
        sys.path.insert(0, _p)

import numpy as np

B, H, W = 8, 512, 512
K = 3
NCORES = 8
# Row-blocked layout: partition p holds output rows 4p..4p+3 (j in 0..3).
# Extended map tiles carry 6 row-variants per partition (jj = j + ky + 1,
# ky in {-1,0,1}) so every vertical shift is a free-dim view.
JB = 4            # rows per partition
JJ = 6            # extended rows per partition (j + ky + 1, 0..5)
AW = 520          # padded image row width
DW = 516          # difference-map row width

_compiled = None


def _build():
    import concourse.bacc as bacc
    import concourse.mybir as mybir
    from concourse.tile import TileContext

    f16, f32 = mybir.dt.float16, mybir.dt.float32
    ALU = mybir.AluOpType
    ACTF = mybir.ActivationFunctionType

    nc = bacc.Bacc("TRN2", target_bir_lowering=False, debug=False,
                   num_devices=NCORES)
    ae_d = nc.dram_tensor("ae", [128, JJ, AW], f16, kind="ExternalInput")
    dxe_d = nc.dram_tensor("dxe", [128, JJ, DW], f16, kind="ExternalInput")
    dye_d = nc.dram_tensor("dye", [128, JJ, DW], f16, kind="ExternalInput")
    dxye_d = nc.dram_tensor("dxye", [128, JJ, DW], f16, kind="ExternalInput")
    off = nc.dram_tensor("off", [2 * K * K, H, W], f16, kind="ExternalInput")
    wdg = nc.dram_tensor("wdg", [128, K * K, 128], f16, kind="ExternalInput")
    out = nc.dram_tensor("out", [H, W], f16, kind="ExternalOutput")

    with TileContext(nc) as tc:
        with (
            tc.tile_pool(name="maps", bufs=1) as mp,
            tc.tile_pool(name="lxy", bufs=6) as lp,
            tc.tile_pool(name="tmp", bufs=2) as tp,
            tc.tile_pool(name="cst", bufs=1) as cp,
            tc.tile_pool(name="psum", bufs=1, space="PSUM") as pp,
        ):
            psum = pp.tile([128, JB, W], f32, name="psum")

            # Loads go out in exact DVE-consumption order, alternating the
            # two HWDGE rings so the stream stays just ahead of compute.
            ring = [nc.sync, nc.scalar]
            rix = [0]

            def dma(dst, src):
                eng = ring[rix[0] & 1]
                rix[0] += 1
                eng.dma_start(out=dst, in_=src)

            def load_ch(k, d):
                # d=1 -> lx (x offsets), d=0 -> ly
                t = lp.tile([128, JB, W], f16, tag=f"l{d}", name=f"l{d}{k}")
                dma(t[:], off[2 * k + d].rearrange("(p j) c -> p j c", j=JB))
                return t

            def map_tile(name, w):
                return mp.tile([128, JJ, w], f16, name=name)

            def load_ch_half(k, d, t, js):
                dma(t[:, js],
                    off[2 * k + d].rearrange("(p j) c -> p j c", j=JB)[:, js])

            # loads split fine-grained at the front so the very first DVE
            # ops wait on as few bytes as possible
            lx0 = lp.tile([128, JB, W], f16, tag="l1", name="lx0")
            ly0 = lp.tile([128, JB, W], f16, tag="l0", name="ly0")
            dxe = map_tile("dxe", DW)
            dxye = map_tile("dxye", DW)
            dye = map_tile("dye", DW)
            load_ch_half(0, 1, lx0, slice(0, 2))
            dma(dxe[:, 0:2], dxe_d[:, 0:2])
            wd = cp.tile([128, K * K, 128], f16, name="wd")
            dma(wd[:], wdg[:])
            dma(dxye[:, 0:2], dxye_d[:, 0:2])
            dma(dye[:, 0:2], dye_d[:, 0:2])
            load_ch_half(0, 0, ly0, slice(0, 2))
            load_ch_half(0, 1, lx0, slice(2, 4))
            dma(dxe[:, 2:4], dxe_d[:, 2:4])
            dma(dxye[:, 2:4], dxye_d[:, 2:4])
            dma(dye[:, 2:4], dye_d[:, 2:4])
            load_ch_half(0, 0, ly0, slice(2, 4))
            ae = map_tile("ae", AW)
            dma(ae[:], ae_d[:])
            lx1 = lp.tile([128, JB, W], f16, tag="l1", name="lx1")
            ly1 = lp.tile([128, JB, W], f16, tag="l0", name="ly1")
            lx2 = lp.tile([128, JB, W], f16, tag="l1", name="lx2")
            ly2 = lp.tile([128, JB, W], f16, tag="l0", name="ly2")
            load_ch_half(1, 1, lx1, slice(0, 2))
            load_ch_half(1, 0, ly1, slice(0, 2))
            load_ch_half(1, 1, lx1, slice(2, 4))
            load_ch_half(1, 0, ly1, slice(2, 4))
            load_ch_half(2, 1, lx2, slice(0, 2))
            load_ch_half(2, 0, ly2, slice(0, 2))
            load_ch_half(2, 1, lx2, slice(2, 4))
            load_ch_half(2, 0, ly2, slice(2, 4))
            dma(dxe[:, 4:6], dxe_d[:, 4:6])
            dma(dxye[:, 4:6], dxye_d[:, 4:6])
            dma(dye[:, 4:6], dye_d[:, 4:6])
            lxs = {0: lx0, 1: lx1, 2: lx2}
            lys = {0: ly0, 1: ly1, 2: ly2}
            for k in range(3, K * K):
                lxs[k] = load_ch(k, 1)
                lys[k] = load_ch(k, 0)

            first = [True] * JB

            def mm(j, wk, mov, stop=False):
                nc.tensor.matmul(psum[:, j, :], wk, mov,
                                 start=first[j], stop=stop)
                first[j] = False

            for k in range(K * K):
                r3, kc = divmod(k, K)
                lx, ly = lxs.pop(k), lys.pop(k)
                wk = wd[:, k, :]
                if k in (0, 1, 2, K * K - 1):
                    # first tap: halves let compute start on fewer bytes;
                    # final tap: halves let banks 0/1 close and drain early
                    for j0, j1 in ((0, 1), (2, 3)):
                        js = slice(j0, j1 + 1)
                        dxv = dxe[:, r3 + j0:r3 + j1 + 1, kc:kc + W]
                        dxyv = dxye[:, r3 + j0:r3 + j1 + 1, kc:kc + W]
                        dyv = dye[:, r3 + j0:r3 + j1 + 1, kc:kc + W]
                        t = tp.tile([128, 2, W], f16, tag="ht", name="ht")
                        t3 = tp.tile([128, 2, W], f16, tag="ht3", name="ht3")
                        s = tp.tile([128, 2, W], f16, tag="hs", name="hs")
                        t2 = tp.tile([128, 2, W], f16, tag="ht2", name="ht2")
                        nc.vector.tensor_tensor(t[:], lx[:, js], dxv, ALU.mult)
                        nc.vector.tensor_tensor(t3[:], lx[:, js], dxyv,
                                                ALU.mult)
                        nc.vector.tensor_tensor(s[:], t3[:], dyv, ALU.add)
                        nc.vector.tensor_tensor(t2[:], ly[:, js], s[:],
                                                ALU.mult)
                        stop = k == K * K - 1
                        if stop:
                            for j in (j0, j1):
                                mm(j, wk, ae[:, j + r3, kc:kc + W])
                        for j in (j0, j1):
                            mm(j, wk, t[:, j - j0, :])
                        for j in (j0, j1):
                            mm(j, wk, t2[:, j - j0, :], stop=stop)
                        if not stop:
                            for j in (j0, j1):
                                mm(j, wk, ae[:, j + r3, kc:kc + W])
                    continue
                if True:
                    dxv = dxe[:, r3:r3 + JB, kc:kc + W]
                    dxyv = dxye[:, r3:r3 + JB, kc:kc + W]
                    dyv = dye[:, r3:r3 + JB, kc:kc + W]
                    t = tp.tile([128, JB, W], f16, tag="t", name="t", bufs=3)
                    t3 = tp.tile([128, JB, W], f16, tag="t3", name="t3", bufs=3)
                    s = tp.tile([128, JB, W], f16, tag="s", name="s", bufs=3)
                    t2 = tp.tile([128, JB, W], f16, tag="t2", name="t2", bufs=3)
                    nc.vector.tensor_tensor(t[:], lx[:], dxv, ALU.mult)
                    nc.vector.tensor_tensor(t3[:], lx[:], dxyv, ALU.mult)
                    nc.vector.tensor_tensor(s[:], t3[:], dyv, ALU.add)
                    nc.vector.tensor_tensor(t2[:], ly[:], s[:], ALU.mult)
                    for j in range(JB):
                        mm(j, wk, t[:, j, :])
                    for j in range(JB):
                        mm(j, wk, t2[:, j, :])
                    for j in range(JB):
                        mm(j, wk, ae[:, j + r3, kc:kc + W])
            # tail: per-bank copies alternate DVE/ACT and drain as four
            # pipelined output DMAs on the two rings.
            res = cp.tile([128, JB, W], f16, name="res")
            outv = out.rearrange("(p j) c -> p j c", j=JB)
            nc.vector.tensor_copy(res[:, 0], psum[:, 0, :])
            nc.scalar.activation(res[:, 2], psum[:, 2, :], ACTF.Copy)
            nc.sync.dma_start(out=outv[:, 0], in_=res[:, 0])
            nc.scalar.dma_start(out=outv[:, 2], in_=res[:, 2])
            nc.vector.tensor_copy(res[:, 1], psum[:, 1, :])
            nc.scalar.activation(res[:, 3], psum[:, 3, :], ACTF.Copy)
            nc.sync.dma_start(out=outv[:, 1], in_=res[:, 1])
            nc.scalar.dma_start(out=outv[:, 3], in_=res[:, 3])

    nc.compile()
    return nc


def kernel(input, weight, offset):
    global _compiled
    from concourse.bass_utils import run_bass_kernel_spmd

    if _compiled is None:
        _compiled = _build()
    nc = _compiled

    input = np.asarray(input, dtype=np.float32)
    offset = np.asarray(offset, dtype=np.float32)
    w9 = np.asarray(weight, dtype=np.float32).reshape(K * K)
    wdg = np.zeros((128, K * K, 128), np.float16)
    idx = np.arange(128)
    for k in range(K * K):
        wdg[idx, k, idx] = w9[k].astype(np.float16)

    jj_rows = 4 * np.arange(128)[:, None] + np.arange(JJ)[None, :]

    in_maps = []
    for b in range(B):
        ipad = np.zeros((515, AW), np.float32)
        ipad[1:H + 1, 1:W + 1] = input[b]
        dx = ipad[:, 1:] - ipad[:, :-1]          # [515, 519]
        dy = ipad[1:, :] - ipad[:-1, :]          # [514, 520]
        dxy = dy[:, 1:] - dy[:, :-1]             # [514, 519]
        ae = ipad.astype(np.float16)[jj_rows]                  # [128,6,520]
        dxe = np.ascontiguousarray(dx[:, :DW].astype(np.float16)[jj_rows])
        dye = np.ascontiguousarray(dy[:, :DW].astype(np.float16)[jj_rows])
        dxye = np.ascontiguousarray(dxy[:, :DW].astype(np.float16)[jj_rows])
        offh = np.ascontiguousarray(offset[b].astype(np.float16))
        in_maps.append({
            "ae": np.ascontiguousarray(ae),
            "dxe": dxe, "dye": dye, "dxye": dxye,
            "off": offh, "wdg": wdg,
        })

    res = run_bass_kernel_spmd(nc, in_maps, list(range(NCORES)), trace=False)
    return np.stack([res.results[b]["out"] for b in range(B)],
                    axis=0).astype(np.float32)
